# revision 1
# baseline (speedup 1.0000x reference)
"""Trainium2 Bass kernel: 3-layer KAN forward pass.

Network (per batch row n of 4096):
  h0 = interleave(xs, ys)                          (4096, 1023)
  h_{l+1} = relu(h_l) @ base_w_l.T + einsum('nfk,ofk->no', B(h_l), spline_w_l*scaler_l)
  out = h3                                         (4096, 512)

Device algorithm:
  - Data parallel over batch: core c computes rows [c*512, (c+1)*512).
  - Activations stay transposed on-chip: features on SBUF partitions,
    batch on the free dim, so each layer's PSUM output feeds the next
    layer with zero transposes.
  - Each layer = 9 accumulating matmul channels per output chunk:
    channel 0 is relu(x), channels 1..8 are the cubic B-spline bases.
  - Bases via closed form (knots at t = (x+2.2)/0.4 integers):
      b_j(x) = (relu(2-d)^3 - 4*relu(1-d)^3)/6,  d = |2.5x + 3.5 - j|
    computed as min/square/mult ops spread across ACT/DVE/GPSIMD; the
    1/6 is folded into the host-prepped spline weights.
  - float32r matmuls (full PE rate at N=512, ~1.5e-4 rms rounding).
  - h ping-pongs through DRAM tiles between layers; PSUM partials are
    accumulated into SBUF-resident per-output-chunk tiles across
    f-groups of 2 contraction chunks.
"""
import numpy as np
import concourse.bass as bass
import concourse.mybir as mybir
import concourse.tile as tile
from concourse import bacc
from concourse.bass_utils import run_bass_kernel_spmd

F32 = mybir.dt.float32
F32R = mybir.dt.float32r
ALU = mybir.AluOpType
AFT = mybir.ActivationFunctionType

N_CORES = 8
BATCH = 4096
POINTS = 512
NB = BATCH // N_CORES                     # 512 batch rows per core
IN0 = 2 * POINTS - 1                      # 1023
LAYER_DIMS = [(1024, 2048), (2048, 2048), (2048, 512)]  # (F padded, O)
GROUP_FC = 2                              # f-chunks per PSUM accumulation group
NCH = 9                                   # 1 base + 8 spline channels

_CACHE = {}


def build_nc():
    nc = bacc.Bacc("TRN2", target_bir_lowering=False, debug=False)
    h0 = nc.dram_tensor("h0", [LAYER_DIMS[0][0], NB], F32, kind="ExternalInput")
    ws = [nc.dram_tensor(f"w{l}", [O // 128, F // 128, 128, NCH * 128], F32R,
                         kind="ExternalInput")
          for l, (F, O) in enumerate(LAYER_DIMS)]
    out = nc.dram_tensor("out", [LAYER_DIMS[2][1], NB], F32,
                         kind="ExternalOutput")

    with tile.TileContext(nc) as tc:
        with (tc.tile_pool(name="xp", bufs=4) as xp,
              tc.tile_pool(name="ap", bufs=3) as ap,
              tc.tile_pool(name="dp", bufs=2) as dp,
              tc.tile_pool(name="tp", bufs=1) as tp,
              tc.tile_pool(name="wp", bufs=5) as wp,
              tc.tile_pool(name="hp", bufs=1) as hp,
              tc.tile_pool(name="pp", bufs=6, space="PSUM") as pp,
              tc.tile_pool(name="dr", bufs=2, space="DRAM") as dr):

            bias = xp.tile([128, 8], F32, tag="bias")
            for j in range(8):
                nc.vector.memset(bias[:, j:j + 1], 3.5 - j)

            hd_prev = None
            for l, (F, O) in enumerate(LAYER_DIMS):
                n_f, n_o = F // 128, O // 128
                hn = [None] * n_o
                for g in range(n_f // GROUP_FC):
                    fcs = list(range(g * GROUP_FC, (g + 1) * GROUP_FC))
                    acts = {}
                    for fc in fcs:
                        x = xp.tile([128, NB], F32, tag="x")
                        if l == 0:
                            nc.sync.dma_start(x[:], h0[fc * 128:(fc + 1) * 128, :])
                        else:
                            nc.sync.dma_start(x[:], hd_prev[fc][:])
                        a = ap.tile([128, NCH * NB], F32R, tag="acts")
                        nc.scalar.activation(a[:, :NB], x[:], AFT.Relu)
                        for h in range(2):
                            d = dp.tile([128, 4 * NB], F32, tag="d")
                            for idx in range(4):
                                j = 4 * h + idx
                                nc.scalar.activation(
                                    d[:, idx * NB:(idx + 1) * NB], x[:], AFT.Abs,
                                    bias=bias[:, j:j + 1], scale=2.5)
                            m1 = tp.tile([128, 4 * NB], F32, tag="m1")
                            nc.gpsimd.tensor_scalar(m1[:], d[:], 2.0, 0.0,
                                                    ALU.subtract, ALU.min)
                            m2 = tp.tile([128, 4 * NB], F32, tag="m2")
                            nc.gpsimd.tensor_scalar(m2[:], d[:], 1.0, 0.0,
                                                    ALU.subtract, ALU.min)
                            q1 = tp.tile([128, 4 * NB], F32, tag="q1")
                            nc.scalar.activation(q1[:], m1[:], AFT.Square)
                            q2 = tp.tile([128, 4 * NB], F32, tag="q2")
                            nc.scalar.activation(q2[:], m2[:], AFT.Square)
                            # cubes in place: q1 <- q1*m1 = -relu(2-d)^3 etc.
                            nc.vector.tensor_tensor(q1[:], q1[:], m1[:], ALU.mult)
                            nc.gpsimd.tensor_tensor(q2[:], q2[:], m2[:], ALU.mult)
                            # bb = 4*q2 - q1 = relu(2-d)^3 - 4*relu(1-d)^3 = 6*B3
                            nc.vector.scalar_tensor_tensor(
                                a[:, (1 + 4 * h) * NB:(5 + 4 * h) * NB],
                                q2[:], 4.0, q1[:], ALU.mult, ALU.subtract)
                        acts[fc] = a
                    for oc in range(n_o):
                        w = wp.tile([128, GROUP_FC * NCH * 128], F32R, tag="w")
                        for i in range(GROUP_FC):
                            nc.sync.dma_start(
                                w[:, i * NCH * 128:(i + 1) * NCH * 128],
                                ws[l][oc, fcs[0] + i, :, :])
                        ps = pp.tile([128, NB], F32, tag="ps")
                        k, klast = 0, GROUP_FC * NCH - 1
                        for i, fc in enumerate(fcs):
                            for ch in range(NCH):
                                nc.tensor.matmul(
                                    ps[:],
                                    w[:, (i * NCH + ch) * 128:(i * NCH + ch + 1) * 128],
                                    acts[fc][:, ch * NB:(ch + 1) * NB],
                                    start=(k == 0), stop=(k == klast))
                                k += 1
                        if g == 0:
                            t = hp.tile([128, NB], F32, tag=f"hn{oc}")
                            hn[oc] = t
                            nc.vector.tensor_copy(t[:], ps[:])
                        else:
                            nc.vector.tensor_tensor(hn[oc][:], ps[:], hn[oc][:],
                                                    ALU.add)
                if l < 2:
                    hd = [None] * n_o
                    for oc in range(n_o):
                        hdt = dr.tile([128, NB], F32, tag=f"hd{oc}")
                        nc.sync.dma_start(hdt[:], hn[oc][:])
                        hd[oc] = hdt
                    hd_prev = hd
                else:
                    for oc in range(n_o):
                        nc.sync.dma_start(out[oc * 128:(oc + 1) * 128, :],
                                          hn[oc][:])
    nc.compile()
    return nc


def _prep_weights(base_w, spline_w, scaler, F_pad, O):
    """(O,Fin) base + (O,Fin,8) spline*scaler -> tiled (n_o, n_f, 128, 9*128).

    Channel 0 is the base weight; channels 1..8 are spline_w*scaler/6
    (the kernel computes 6*B3). Element [oc, fc, f, ch*128+o] =
    W[ch, fc*128+f, oc*128+o] with W in (ch, F_pad, O) layout.
    """
    Fin = base_w.shape[1]
    n_f, n_o = F_pad // 128, O // 128
    W_all = np.zeros((NCH, F_pad, O), np.float32)
    W_all[0, :Fin, :] = base_w.T
    sw = (spline_w * scaler[:, :, None]) * np.float32(1.0 / 6.0)
    W_all[1:, :Fin, :] = sw.transpose(2, 1, 0)
    wt = W_all.reshape(NCH, n_f, 128, n_o, 128).transpose(3, 1, 2, 0, 4)
    return np.ascontiguousarray(wt).reshape(n_o, n_f, 128, NCH * 128)


def kernel(xs, ys, base_w0, spline_w0, scaler0, base_w1, spline_w1, scaler1,
           base_w2, spline_w2, scaler2):
    xs = np.asarray(xs, np.float32)
    ys = np.asarray(ys, np.float32)
    weights = [(np.asarray(base_w0, np.float32), np.asarray(spline_w0, np.float32),
                np.asarray(scaler0, np.float32)),
               (np.asarray(base_w1, np.float32), np.asarray(spline_w1, np.float32),
                np.asarray(scaler1, np.float32)),
               (np.asarray(base_w2, np.float32), np.asarray(spline_w2, np.float32),
                np.asarray(scaler2, np.float32))]

    if "nc" not in _CACHE:
        _CACHE["nc"] = build_nc()
    nc = _CACHE["nc"]

    # h0 = interleave(x0,y0,x1,y1,...,x510,y510,x511), transposed + padded
    xs2 = xs[:, :, 0]
    inter = np.stack([xs2[:, :-1], ys[:, :-1]], axis=-1).reshape(BATCH, -1)
    h0 = np.concatenate([inter, xs2[:, -1:]], axis=1)      # (4096, 1023)
    h0T = np.zeros((LAYER_DIMS[0][0], BATCH), np.float32)
    h0T[:IN0, :] = h0.T

    w_t = [_prep_weights(*weights[l], LAYER_DIMS[l][0], LAYER_DIMS[l][1])
           for l in range(3)]

    in_maps = [{"h0": np.ascontiguousarray(h0T[:, c * NB:(c + 1) * NB]),
                "w0": w_t[0], "w1": w_t[1], "w2": w_t[2]}
               for c in range(N_CORES)]
    res = run_bass_kernel_spmd(nc, in_maps, core_ids=list(range(N_CORES)))

    out = np.empty((BATCH, POINTS), np.float32)
    for c in range(N_CORES):
        out[c * NB:(c + 1) * NB, :] = res.results[c]["out"].T
    return out



# revision 2
# speedup vs baseline: 9.9922x; 9.9922x over previous
"""Trainium2 Bass kernel v2: 3-layer KAN forward pass (bf16 matmul path).

Per-core (data parallel over batch, NB=512 rows/core):
  h0 = interleave(xs, ys) transposed: features on partitions, batch on free.
  Layer l: acts = [relu(x), 6*B_j(x) for active j] as bf16;
           out[oc] = sum over f-chunks/channels of w[oc,f,ch]^T @ acts[f,ch]
  Bases via closed form: 6*B_j(x) = relu(2-d)^3 - 4*relu(1-d)^3,
  d = |2.5x + 3.5 - j|; the 1/6 is folded into host-prepped weights.
  Layer 0 inputs lie in [0,1) so bases j=0,1 are identically zero and
  their channels are dropped (7 channels instead of 9).

vs v1: bf16 weights+acts (half DMA, same PE rate), no DRAM bounce between
layers (hidden activations stay in SBUF), one coalesced weight DMA per
(out-chunk, f-group) via a [n_o, 128, n_f*NCH*128] layout, elementwise
work balanced across ACT/DVE/GPSIMD.
"""
import numpy as np
import concourse.bass as bass
import concourse.mybir as mybir
import concourse.tile as tile
from concourse import bacc
from concourse.bass_utils import run_bass_kernel_spmd

F32 = mybir.dt.float32
BF16 = mybir.dt.bfloat16
ALU = mybir.AluOpType
AFT = mybir.ActivationFunctionType

N_CORES = 8
BATCH = 4096
POINTS = 512
NB = BATCH // N_CORES                     # 512 batch rows per core
IN0 = 2 * POINTS - 1                      # 1023
LAYER_DIMS = [(1024, 2048), (2048, 2048), (2048, 512)]  # (F padded, O)
LAYER_JS = [list(range(2, 8)), list(range(8)), list(range(8))]
GROUP_FC = 2

_CACHE = {}


def _emit_acts(nc, a, x, js, bias, dp, tp):
    """Write [relu(x), 6*B_j(x) for j in js] into acts tile a (bf16).

    Engine balance per chunk (S=8): ACT 17u (abs + q1-square + relu),
    GPSIMD 16u (two min-thresholds), DVE 32u (q2 square, cubes, combine).
    """
    S = len(js)
    H = S // 2
    nc.scalar.activation(a[:, :NB], x[:], AFT.Relu)
    for h in range(2):
        js_h = js[h * H:(h + 1) * H]
        W = H * NB
        d = dp.tile([128, W], F32, tag="d")
        for idx, j in enumerate(js_h):
            nc.scalar.activation(d[:, idx * NB:(idx + 1) * NB], x[:], AFT.Abs,
                                 bias=bias[:, j:j + 1], scale=2.5)
        m1 = tp.tile([128, W], F32, tag="m1")
        nc.gpsimd.tensor_scalar(m1[:], d[:], 2.0, 0.0, ALU.subtract, ALU.min)
        m2 = tp.tile([128, W], F32, tag="m2")
        nc.gpsimd.tensor_scalar(m2[:], d[:], 1.0, 0.0, ALU.subtract, ALU.min)
        q1 = tp.tile([128, W], F32, tag="q1")
        nc.scalar.activation(q1[:], m1[:], AFT.Square)
        q2 = tp.tile([128, W], F32, tag="q2")
        nc.vector.tensor_tensor(q2[:], m2[:], m2[:], ALU.mult)
        # cubes in place: q1 <- q1*m1 = -relu(2-d)^3, q2 <- q2*m2 = -relu(1-d)^3
        nc.vector.tensor_tensor(q1[:], q1[:], m1[:], ALU.mult)
        nc.vector.tensor_tensor(q2[:], q2[:], m2[:], ALU.mult)
        # a_ch = 4*q2 - q1 = relu(2-d)^3 - 4*relu(1-d)^3 = 6*B3  (bf16 out)
        nc.vector.scalar_tensor_tensor(
            a[:, (1 + h * H) * NB:(1 + (h + 1) * H) * NB],
            q2[:], 4.0, q1[:], ALU.mult, ALU.subtract)


def build_nc(repeat=1):
    nc = bacc.Bacc("TRN2", target_bir_lowering=False, debug=False)
    h0 = nc.dram_tensor("h0", [LAYER_DIMS[0][0], NB], F32, kind="ExternalInput")
    ws = []
    for l, (F, O) in enumerate(LAYER_DIMS):
        nch = 1 + len(LAYER_JS[l])
        ws.append(nc.dram_tensor(f"w{l}", [O // 128, 128, (F // 128) * nch * 128],
                                 BF16, kind="ExternalInput"))
    out = nc.dram_tensor("out", [LAYER_DIMS[2][1], NB], F32,
                         kind="ExternalOutput")

    with tile.TileContext(nc) as tc:
        with (tc.tile_pool(name="xp", bufs=4) as xp,
              tc.tile_pool(name="ap", bufs=4) as ap,
              tc.tile_pool(name="dp", bufs=2) as dp,
              tc.tile_pool(name="tp", bufs=1) as tp,
              tc.tile_pool(name="wp", bufs=6) as wp,
              tc.tile_pool(name="ha", bufs=1) as ha,
              tc.tile_pool(name="hb", bufs=1) as hb,
              tc.tile_pool(name="pp", bufs=6, space="PSUM") as pp):

            bias = xp.tile([128, 8], F32, tag="bias")
            for j in range(8):
                nc.vector.memset(bias[:, j:j + 1], 3.5 - j)

            for _rep in range(repeat):
                hn_prev = None
                for l, (F, O) in enumerate(LAYER_DIMS):
                    js = LAYER_JS[l]
                    nch = 1 + len(js)
                    n_f, n_o = F // 128, O // 128
                    hpool = ha if l % 2 == 0 else hb
                    hn = [None] * n_o
                    for g in range(n_f // GROUP_FC):
                        fcs = list(range(g * GROUP_FC, (g + 1) * GROUP_FC))
                        acts = {}
                        for fc in fcs:
                            if l == 0:
                                x = xp.tile([128, NB], F32, tag="x")
                                nc.sync.dma_start(
                                    x[:], h0[fc * 128:(fc + 1) * 128, :])
                            else:
                                x = hn_prev[fc]
                            a = ap.tile([128, nch * NB], BF16, tag="acts")
                            _emit_acts(nc, a, x, js, bias, dp, tp)
                            acts[fc] = a
                        for oc in range(n_o):
                            w = wp.tile([128, GROUP_FC * nch * 128], BF16,
                                        tag="w")
                            c0 = fcs[0] * nch * 128
                            nc.sync.dma_start(
                                w[:], ws[l][oc, :, c0:c0 + GROUP_FC * nch * 128])
                            ps = pp.tile([128, NB], F32, tag="ps")
                            k, klast = 0, GROUP_FC * nch - 1
                            for i, fc in enumerate(fcs):
                                for ch in range(nch):
                                    nc.tensor.matmul(
                                        ps[:],
                                        w[:, (i * nch + ch) * 128:
                                          (i * nch + ch + 1) * 128],
                                        acts[fc][:, ch * NB:(ch + 1) * NB],
                                        start=(k == 0), stop=(k == klast))
                                    k += 1
                            if g == 0:
                                t = hpool.tile([128, NB], F32, tag=f"hn{oc}")
                                hn[oc] = t
                                nc.vector.tensor_copy(t[:], ps[:])
                            else:
                                nc.vector.tensor_tensor(hn[oc][:], ps[:],
                                                        hn[oc][:], ALU.add)
                    if l == 2:
                        for oc in range(n_o):
                            nc.sync.dma_start(out[oc * 128:(oc + 1) * 128, :],
                                              hn[oc][:])
                    hn_prev = hn
    nc.compile()
    return nc


def _prep_weights(base_w, spline_w, scaler, F_pad, O, js):
    """-> [n_o, 128, n_f * nch * 128] bf16.

    Channel 0 is the base weight; channels 1.. are spline_w*scaler/6 for
    basis indices js (the kernel computes 6*B3). Element
    [oc, p, (fc*nch + ch)*128 + o] = W[ch, fc*128+p, oc*128+o].
    """
    Fin = base_w.shape[1]
    n_f, n_o = F_pad // 128, O // 128
    nch = 1 + len(js)
    W_all = np.zeros((nch, F_pad, O), np.float32)
    W_all[0, :Fin, :] = base_w.T
    sw = (spline_w * scaler[:, :, None]) * np.float32(1.0 / 6.0)
    for i, j in enumerate(js):
        W_all[1 + i, :Fin, :] = sw[:, :, j].T
    wt = W_all.reshape(nch, n_f, 128, n_o, 128).transpose(3, 2, 1, 0, 4)
    wt = np.ascontiguousarray(wt).reshape(n_o, 128, n_f * nch * 128)
    return wt.astype(mybir.dt.np(BF16))


def kernel(xs, ys, base_w0, spline_w0, scaler0, base_w1, spline_w1, scaler1,
           base_w2, spline_w2, scaler2):
    xs = np.asarray(xs, np.float32)
    ys = np.asarray(ys, np.float32)
    weights = [(np.asarray(base_w0, np.float32), np.asarray(spline_w0, np.float32),
                np.asarray(scaler0, np.float32)),
               (np.asarray(base_w1, np.float32), np.asarray(spline_w1, np.float32),
                np.asarray(scaler1, np.float32)),
               (np.asarray(base_w2, np.float32), np.asarray(spline_w2, np.float32),
                np.asarray(scaler2, np.float32))]

    if "nc" not in _CACHE:
        _CACHE["nc"] = build_nc()
    nc = _CACHE["nc"]

    # h0 = interleave(x0,y0,...,x510,y510,x511), transposed + zero-padded
    xs2 = xs[:, :, 0]
    inter = np.stack([xs2[:, :-1], ys[:, :-1]], axis=-1).reshape(BATCH, -1)
    h0 = np.concatenate([inter, xs2[:, -1:]], axis=1)      # (4096, 1023)
    h0T = np.zeros((LAYER_DIMS[0][0], BATCH), np.float32)
    h0T[:IN0, :] = h0.T

    w_t = [_prep_weights(*weights[l], LAYER_DIMS[l][0], LAYER_DIMS[l][1],
                         LAYER_JS[l]) for l in range(3)]

    in_maps = [{"h0": np.ascontiguousarray(h0T[:, c * NB:(c + 1) * NB]),
                "w0": w_t[0], "w1": w_t[1], "w2": w_t[2]}
               for c in range(N_CORES)]
    res = run_bass_kernel_spmd(nc, in_maps, core_ids=list(range(N_CORES)))

    out = np.empty((BATCH, POINTS), np.float32)
    for c in range(N_CORES):
        out[c * NB:(c + 1) * NB, :] = res.results[c]["out"].T
    return out
